# revision 2
# baseline (speedup 1.0000x reference)
"""Depthwise 3x3 conv + sync BatchNorm (train mode) + ReLU6 on 8 Trainium2 cores.

Sharding: channels (192) split 24-per-core. Depthwise conv and BN are
per-channel independent, so no cross-core communication is needed.

v2: fp16 I/O to halve HBM traffic (the v1 bottleneck: DMA active 93%),
input DMAs split across both HWDGE rings (sync + scalar), H-unpadded x
slab (conv H-boundary handling absorbed into the banded A matrices so
K=112), fp16 y/z for 2x DVE throughput.

Per-channel device pipeline (each core, 24 channels):
  - DMA in: W-padded x slab [112, 32, 114] fp16 (H in partitions).
  - Conv as banded matmuls: for each W-tap dj, lhsT A_dj[k, m] = w[k-m+1, dj]
    (3-diagonal band, edges clipped). 8 image-groups of 4 accumulate 3 taps
    each in PSUM ([112, 448] = one bank per group).
  - ScalarE drains PSUM->SBUF as fp16.
  - DVE bn_stats (fp16, 2x rate) -> bn_aggr -> partition-collapse via
    ones-matmul -> scalar mean/var chain -> per-channel
    scale' = gamma*rsqrt(var+eps), bias' = beta - mean*scale' (conv bias b
    cancels exactly in train-mode BN, so it is never applied).
  - Broadcast (outer-product matmul) scale'/bias' to [112,1]; ScalarE applies
    Relu(scale'*y + bias'); DVE clamps to 6.0 (fp16, 2x); DMA out fp16.
"""

import numpy as np
from contextlib import ExitStack

import concourse.bass as bass
import concourse.mybir as mybir
import concourse.tile as tile
from concourse import bacc, bass_utils

FP32 = mybir.dt.float32
FP16 = mybir.dt.float16
AF = mybir.ActivationFunctionType
ALU = mybir.AluOpType

N, C, H, W = 32, 192, 112, 112
NCORES = 8
CPC = C // NCORES          # 24 channels per core
WP = W + 2                 # zero-padded width only (H handled in banded A)
G = 8                      # image groups (PSUM banks) per channel
IPG = N // G               # 4 images per group
NF = IPG * W               # 448 matmul free dim (fp32 PSUM bank limit 512)
BN_EPS = 1e-5


def _emit(ctx: ExitStack, tc, nc, x_d, a_d, gb_d, o_d, n_ch):
    a_pool = ctx.enter_context(tc.tile_pool(name="a", bufs=1))
    const_pool = ctx.enter_context(tc.tile_pool(name="const", bufs=1))
    x_pool = ctx.enter_context(tc.tile_pool(name="x", bufs=3))
    y_pool = ctx.enter_context(tc.tile_pool(name="y", bufs=4))
    z_pool = ctx.enter_context(tc.tile_pool(name="z", bufs=2))
    st_pool = ctx.enter_context(tc.tile_pool(name="st", bufs=3))
    sc_pool = ctx.enter_context(tc.tile_pool(name="sc", bufs=3))
    st = {c: {} for c in range(n_ch)}
    psum_y = ctx.enter_context(tc.tile_pool(name="py", bufs=5, space="PSUM"))
    psum_s = ctx.enter_context(tc.tile_pool(name="ps", bufs=2, space="PSUM"))
    psum_b = ctx.enter_context(tc.tile_pool(name="pb", bufs=1, space="PSUM"))

    a_all = a_pool.tile([H, n_ch, 3, W], FP16)
    nc.sync.dma_start(a_all[:], a_d.ap())
    gb = const_pool.tile([1, 2 * n_ch], FP32)
    nc.sync.dma_start(gb[:], gb_d.ap())
    ones_col = const_pool.tile([H, 1], FP32)   # lhsT for partition collapse
    nc.vector.memset(ones_col[:], 1.0)
    ones_row = const_pool.tile([1, H], FP32)   # lhsT for partition broadcast
    nc.vector.memset(ones_row[:], 1.0)
    eps_t = const_pool.tile([1, 1], FP32)      # BN eps as Sqrt bias operand
    nc.vector.memset(eps_t[:], BN_EPS)

    def emit_conv(c):
        x_t = x_pool.tile([H, N, WP], FP16)
        # alternate input DMAs between the two HWDGE rings (sync / scalar)
        ring = nc.sync if (c % 2 == 0) else nc.scalar
        ring.dma_start(x_t[:], x_d.ap()[c])
        y_sb = y_pool.tile([H, G, NF], FP16)
        bst = st_pool.tile([H, G, 6], FP32, tag="bst")
        for g in range(G):
            pt = psum_y.tile([H, NF], FP32, tag="pt")
            for dj in range(3):
                nc.tensor.matmul(
                    pt[:],
                    a_all[:, c, dj, :],
                    x_t[:, g * IPG:(g + 1) * IPG, dj:dj + W],
                    start=(dj == 0),
                    stop=(dj == 2),
                )
            nc.scalar.activation(y_sb[:, g, :], pt[:], AF.Copy, bias=0.0)
            nc.vector.bn_stats(bst[:, g, :], y_sb[:, g, :])
        stats3 = st_pool.tile([H, 3], FP32, tag="stats3")
        nc.vector.bn_aggr(stats3[:, 0:2], bst[:])
        nc.vector.tensor_scalar(
            stats3[:, 2:3], stats3[:, 0:1], stats3[:, 0:1], None, op0=ALU.mult
        )
        st[c].update(y=y_sb, stats3=stats3)

    def emit_fin1(c):
        # partition collapse + per-channel scalar chain -> scpair
        stats3 = st[c]["stats3"]
        pst = psum_s.tile([1, 3], FP32, tag="pst")
        nc.tensor.matmul(pst[:], ones_col[:], stats3[:])
        em = sc_pool.tile([1, 3], FP32, tag="em")
        nc.vector.tensor_scalar_mul(em[:], pst[:], 1.0 / H)
        m2 = sc_pool.tile([1, 1], FP32, tag="m2")
        nc.vector.tensor_scalar(m2[:], em[:, 0:1], em[:, 0:1], None, op0=ALU.mult)
        varr = sc_pool.tile([1, 1], FP32, tag="varr")
        nc.vector.tensor_scalar(
            varr[:], em[:, 1:2], em[:, 2:3], m2[:], op0=ALU.add, op1=ALU.subtract
        )
        std = sc_pool.tile([1, 1], FP32, tag="std")
        nc.scalar.activation(std[:], varr[:], AF.Sqrt, bias=eps_t[:])
        istd = sc_pool.tile([1, 1], FP32, tag="istd")
        nc.vector.reciprocal(istd[:], std[:])
        scpair = sc_pool.tile([1, 2], FP32, tag="scpair")
        nc.vector.tensor_scalar(
            scpair[:, 0:1], istd[:], gb[:, c:c + 1], None, op0=ALU.mult
        )
        msc = sc_pool.tile([1, 1], FP32, tag="msc")
        nc.vector.tensor_scalar(
            msc[:], em[:, 0:1], scpair[:, 0:1], None, op0=ALU.mult
        )
        nc.vector.tensor_scalar(
            scpair[:, 1:2], gb[:, n_ch + c:n_ch + c + 1], msc[:], None,
            op0=ALU.subtract,
        )
        st[c]["scpair"] = scpair

    def emit_fin2(c):
        # broadcast scale'/bias' across partitions (outer product)
        pb = psum_b.tile([H, 2], FP32, tag="pb")
        nc.tensor.matmul(pb[:], ones_row[:], st[c]["scpair"][:])
        bc = sc_pool.tile([H, 2], FP32, tag="bc")
        nc.vector.tensor_copy(bc[:], pb[:])
        st[c]["bc"] = bc

    def emit_out(c):
        y_sb, bc = st[c]["y"], st[c]["bc"]
        z_sb = z_pool.tile([H, G, IPG, W], FP16, tag="z")
        hg = G // 2
        for h2 in range(2):
            zf = z_sb[:, h2 * hg:(h2 + 1) * hg].rearrange("p g i w -> p (g i w)")
            nc.scalar.activation(
                zf,
                y_sb[:, h2 * hg:(h2 + 1) * hg, :].rearrange("p g f -> p (g f)"),
                AF.Relu, bias=bc[:, 1:2], scale=bc[:, 0:1],
            )
            nc.vector.tensor_scalar_min(zf, zf, 6.0)
            # SWDGE ring: keeps the HWDGE rings free for x prefetches
            nc.gpsimd.dma_start(
                o_d.ap()[c].rearrange("h (s n) w -> h s n w", s=2)[:, h2],
                z_sb[:, h2 * hg:(h2 + 1) * hg],
            )

    # software pipeline: PE stream is [fin1(c-1) mm, fin2(c-2) mm, conv(c) mms]
    # so every PE instruction is dep-ready when reached (no in-order stalls)
    for c in range(n_ch):
        if c >= 1:
            emit_fin1(c - 1)
        if c >= 2:
            emit_fin2(c - 2)
        emit_conv(c)
        if c >= 3:
            emit_out(c - 3)
    emit_fin1(n_ch - 1)
    for c in range(max(0, n_ch - 2), n_ch):
        emit_fin2(c)
    for c in range(max(0, n_ch - 3), n_ch):
        emit_out(c)


def build_program(n_ch=CPC, enable_asserts=False):
    nc = bacc.Bacc(
        "TRN2",
        debug=False,
        enable_asserts=enable_asserts,
        target_bir_lowering=False,
        num_devices=NCORES,
    )
    x_d = nc.dram_tensor("x", (n_ch, H, N, WP), FP16, kind="ExternalInput")
    a_d = nc.dram_tensor("a", (H, n_ch, 3, W), FP16, kind="ExternalInput")
    gb_d = nc.dram_tensor("gb", (1, 2 * n_ch), FP32, kind="ExternalInput")
    o_d = nc.dram_tensor("o", (n_ch, H, N, W), FP16, kind="ExternalOutput")
    with tile.TileContext(nc) as tc:
        with ExitStack() as ctx:
            _emit(ctx, tc, nc, x_d, a_d, gb_d, o_d, n_ch)
    nc.compile()
    return nc


def make_core_inputs(inputs, w, gamma, beta, k, n_ch=CPC):
    """Host-side shard prep for core k: padded x slab, banded A matrices, gamma/beta."""
    ch = slice(k * n_ch, (k + 1) * n_ch)
    xk = np.zeros((n_ch, H, N, WP), np.float16)
    xk[:, :, :, 1:1 + W] = np.asarray(inputs[:, ch]).transpose(1, 2, 0, 3)
    wk = np.asarray(w[ch]).astype(np.float32)          # (n_ch, 1, 3, 3)
    ak = np.zeros((n_ch, 3, H, W), np.float32)
    m = np.arange(W)
    for di in range(3):
        # out[m] += w[di, dj] * x[m + di - 1]  ->  A[c, dj, m+di-1, m] = w[c, 0, di, dj]
        sel = (m + di - 1 >= 0) & (m + di - 1 < H)
        ak[:, :, m[sel] + di - 1, m[sel]] = wk[:, 0, di, :][:, :, None]
    ak = np.ascontiguousarray(ak.transpose(2, 0, 1, 3)).astype(np.float16)  # (H, n_ch, 3, W)
    gbk = np.concatenate(
        [np.asarray(gamma[ch]), np.asarray(beta[ch])]
    ).astype(np.float32).reshape(1, 2 * n_ch)
    return {"x": xk, "a": ak, "gb": gbk}


_PROGRAM = None


def kernel(inputs, w, b, gamma, beta):
    global _PROGRAM
    if _PROGRAM is None:
        _PROGRAM = build_program()
    inputs = np.asarray(inputs, np.float32)
    in_maps = [make_core_inputs(inputs, w, gamma, beta, k) for k in range(NCORES)]
    res = bass_utils.run_bass_kernel_spmd(_PROGRAM, in_maps, list(range(NCORES)))
    out = np.empty((N, C, H, W), np.float32)
    for k in range(NCORES):
        # per-core output is (CPC, H, N, W) fp16
        out[:, k * CPC:(k + 1) * CPC] = res.results[k]["o"].transpose(2, 0, 1, 3)
    return out


# revision 3
# speedup vs baseline: 1.1590x; 1.1590x over previous
"""Depthwise 3x3 conv + sync BatchNorm (train mode) + ReLU6 on 8 Trainium2 cores.

Sharding: channels (192) split 24-per-core. Depthwise conv and BN are
per-channel independent, so no cross-core communication is needed.

v3: engine rebalance on top of v2's fp16 I/O.
  - ScalarE drain (PSUM->SBUF fp16) carries accum_out => per-partition
    sum(y) for free (per group column of s1).
  - sum(y^2) via DVE scalar_tensor_tensor (y*1)*y with accum_out: lowers to
    InstTensorScalarPtr which runs at 4 elem/cycle for packed fp16 SBUF
    operands (scalar/accum operands exempt from the mode check).
  - BN apply + ReLU6 as two dual-op DVE tensor_scalar instructions at 4x:
    t = (y*scale')+bias' ; z = min(max(t,0),6). No scalar finals, no
    bn_stats, no separate min6.
  - Stats collapse: [s1(8 cols) | ssq(2 cols)] -> ones-matmul -> [1,10] ->
    two tiny free-dim reduces -> mean/var chain -> scale'/bias' broadcast
    via outer-product matmul.

Per-channel device pipeline (each core, 24 channels):
  - DMA in (alternating sync/scalar HWDGE rings): W-padded x slab
    [112, 32, 114] fp16 (H in partitions).
  - Conv as banded matmuls: for each W-tap dj, lhsT A_dj[k, m] = w[k-m+1, dj]
    (3-diagonal band, H edges clipped in A). 8 image-groups of 4 accumulate
    3 taps each in PSUM ([112, 448] = one bank per group).
  - conv bias b cancels exactly in train-mode BN, so it is never applied.
  - Output z fp16 via gpsimd SWDGE ring.
"""

import numpy as np
from contextlib import ExitStack

import concourse.bass as bass
import concourse.mybir as mybir
import concourse.tile as tile
from concourse import bacc, bass_utils

FP32 = mybir.dt.float32
FP16 = mybir.dt.float16
AF = mybir.ActivationFunctionType
ALU = mybir.AluOpType

N, C, H, W = 32, 192, 112, 112
NCORES = 8
CPC = C // NCORES          # 24 channels per core
WP = W + 2                 # zero-padded width only (H handled in banded A)
G = 8                      # image groups (PSUM banks) per channel
IPG = N // G               # 4 images per group
NF = IPG * W               # 448 matmul free dim (fp32 PSUM bank limit 512)
NTOT = N * H * W           # BN reduction size per channel
BN_EPS = 1e-5


def _emit(ctx: ExitStack, tc, nc, x_d, a_d, gb_d, o_d, n_ch):
    a_pool = ctx.enter_context(tc.tile_pool(name="a", bufs=1))
    const_pool = ctx.enter_context(tc.tile_pool(name="const", bufs=1))
    x_pool = ctx.enter_context(tc.tile_pool(name="x", bufs=3))
    y_pool = ctx.enter_context(tc.tile_pool(name="y", bufs=4))
    z_pool = ctx.enter_context(tc.tile_pool(name="z", bufs=2))
    t_pool = ctx.enter_context(tc.tile_pool(name="t", bufs=2))
    q_pool = ctx.enter_context(tc.tile_pool(name="q", bufs=2))
    st_pool = ctx.enter_context(tc.tile_pool(name="st", bufs=3))
    sc_pool = ctx.enter_context(tc.tile_pool(name="sc", bufs=3))
    st = {c: {} for c in range(n_ch)}
    psum_y = ctx.enter_context(tc.tile_pool(name="py", bufs=5, space="PSUM"))
    psum_s = ctx.enter_context(tc.tile_pool(name="ps", bufs=2, space="PSUM"))
    psum_b = ctx.enter_context(tc.tile_pool(name="pb", bufs=1, space="PSUM"))

    a_all = a_pool.tile([H, n_ch, 3, W], FP16)
    nc.sync.dma_start(a_all[:], a_d.ap())
    gb = const_pool.tile([1, 2 * n_ch], FP32)
    nc.sync.dma_start(gb[:], gb_d.ap())
    ones_col = const_pool.tile([H, 1], FP32)   # lhsT for partition collapse
    nc.vector.memset(ones_col[:], 1.0)
    ones_row = const_pool.tile([1, H], FP32)   # lhsT for partition broadcast
    nc.vector.memset(ones_row[:], 1.0)
    eps_t = const_pool.tile([1, 1], FP32)      # BN eps as Sqrt bias operand
    nc.vector.memset(eps_t[:], BN_EPS)

    def emit_conv(c):
        x_t = x_pool.tile([H, N, WP], FP16)
        # alternate input DMAs between the two HWDGE rings (sync / scalar)
        ring = nc.sync if (c % 2 == 0) else nc.scalar
        ring.dma_start(x_t[:], x_d.ap()[c])
        y_sb = y_pool.tile([H, G, NF], FP16)
        # stats: cols 0..7 = per-group sum(y) (drain accum), 8..9 = sum(y^2)
        stats = st_pool.tile([H, 10], FP32, tag="stats")
        for g in range(G):
            pt = psum_y.tile([H, NF], FP32, tag="pt")
            for dj in range(3):
                nc.tensor.matmul(
                    pt[:],
                    a_all[:, c, dj, :],
                    x_t[:, g * IPG:(g + 1) * IPG, dj:dj + W],
                    start=(dj == 0),
                    stop=(dj == 2),
                )
            nc.scalar.activation(
                y_sb[:, g, :], pt[:], AF.Copy, bias=0.0,
                accum_out=stats[:, g:g + 1],
            )
        hg = G // 2
        for h2 in range(2):
            q = q_pool.tile([H, hg * NF], FP16, tag="q")
            yv = y_sb[:, h2 * hg:(h2 + 1) * hg, :].rearrange("p g f -> p (g f)")
            # q = (y*1)*y = y^2 (discarded); accum = sum(y^2): 4x fp16 path
            nc.vector.scalar_tensor_tensor(
                q[:], yv, 1.0, yv, op0=ALU.mult, op1=ALU.mult,
                accum_out=stats[:, 8 + h2:9 + h2],
            )
        st[c].update(y=y_sb, stats=stats)

    def emit_fin1(c):
        # partition collapse + per-channel scalar chain -> scpair
        stats = st[c]["stats"]
        pst = psum_s.tile([1, 10], FP32, tag="pst")
        nc.tensor.matmul(pst[:], ones_col[:], stats[:])
        sums = sc_pool.tile([1, 2], FP32, tag="sums")
        nc.vector.tensor_reduce(
            sums[:, 0:1], pst[:, 0:8], axis=mybir.AxisListType.X, op=ALU.add
        )
        nc.vector.tensor_reduce(
            sums[:, 1:2], pst[:, 8:10], axis=mybir.AxisListType.X, op=ALU.add
        )
        em = sc_pool.tile([1, 2], FP32, tag="em")   # [mean, E[y^2]]
        nc.vector.tensor_scalar_mul(em[:], sums[:], 1.0 / NTOT)
        m2 = sc_pool.tile([1, 1], FP32, tag="m2")
        nc.vector.tensor_scalar(m2[:], em[:, 0:1], em[:, 0:1], None, op0=ALU.mult)
        varr = sc_pool.tile([1, 1], FP32, tag="varr")
        nc.vector.tensor_scalar(
            varr[:], em[:, 1:2], m2[:], None, op0=ALU.subtract
        )
        std = sc_pool.tile([1, 1], FP32, tag="std")
        nc.scalar.activation(std[:], varr[:], AF.Sqrt, bias=eps_t[:])
        istd = sc_pool.tile([1, 1], FP32, tag="istd")
        nc.vector.reciprocal(istd[:], std[:])
        scpair = sc_pool.tile([1, 2], FP32, tag="scpair")
        nc.vector.tensor_scalar(
            scpair[:, 0:1], istd[:], gb[:, c:c + 1], None, op0=ALU.mult
        )
        msc = sc_pool.tile([1, 1], FP32, tag="msc")
        nc.vector.tensor_scalar(
            msc[:], em[:, 0:1], scpair[:, 0:1], None, op0=ALU.mult
        )
        nc.vector.tensor_scalar(
            scpair[:, 1:2], gb[:, n_ch + c:n_ch + c + 1], msc[:], None,
            op0=ALU.subtract,
        )
        st[c]["scpair"] = scpair

    def emit_fin2(c):
        # broadcast scale'/bias' across partitions (outer product)
        pb = psum_b.tile([H, 2], FP32, tag="pb")
        nc.tensor.matmul(pb[:], ones_row[:], st[c]["scpair"][:])
        bc = sc_pool.tile([H, 2], FP32, tag="bc")
        nc.vector.tensor_copy(bc[:], pb[:])
        st[c]["bc"] = bc

    def emit_out(c):
        y_sb, bc = st[c]["y"], st[c]["bc"]
        z_sb = z_pool.tile([H, G, IPG, W], FP16, tag="z")
        t_sb = t_pool.tile([H, G // 2, IPG, W], FP16, tag="t")
        hg = G // 2
        for h2 in range(2):
            zf = z_sb[:, h2 * hg:(h2 + 1) * hg].rearrange("p g i w -> p (g i w)")
            tf = t_sb.rearrange("p g i w -> p (g i w)")
            yv = y_sb[:, h2 * hg:(h2 + 1) * hg, :].rearrange("p g f -> p (g f)")
            # BN apply + ReLU6, both dual-op tensor_scalar at 4x fp16 rate
            nc.vector.tensor_scalar(
                tf, yv, bc[:, 0:1], bc[:, 1:2], op0=ALU.mult, op1=ALU.add
            )
            nc.vector.tensor_scalar(
                zf, tf, 0.0, 6.0, op0=ALU.max, op1=ALU.min
            )
            # SWDGE ring: keeps the HWDGE rings free for x prefetches
            nc.gpsimd.dma_start(
                o_d.ap()[c].rearrange("h (s n) w -> h s n w", s=2)[:, h2],
                z_sb[:, h2 * hg:(h2 + 1) * hg],
            )

    # software pipeline: PE stream is [fin1(c-1) mm, fin2(c-2) mm, conv(c) mms]
    # so every PE instruction is dep-ready when reached (no in-order stalls)
    for c in range(n_ch):
        if c >= 1:
            emit_fin1(c - 1)
        if c >= 2:
            emit_fin2(c - 2)
        emit_conv(c)
        if c >= 3:
            emit_out(c - 3)
    emit_fin1(n_ch - 1)
    for c in range(max(0, n_ch - 2), n_ch):
        emit_fin2(c)
    for c in range(max(0, n_ch - 3), n_ch):
        emit_out(c)


def build_program(n_ch=CPC, enable_asserts=False):
    nc = bacc.Bacc(
        "TRN2",
        debug=False,
        enable_asserts=enable_asserts,
        target_bir_lowering=False,
        num_devices=NCORES,
    )
    x_d = nc.dram_tensor("x", (n_ch, H, N, WP), FP16, kind="ExternalInput")
    a_d = nc.dram_tensor("a", (H, n_ch, 3, W), FP16, kind="ExternalInput")
    gb_d = nc.dram_tensor("gb", (1, 2 * n_ch), FP32, kind="ExternalInput")
    o_d = nc.dram_tensor("o", (n_ch, H, N, W), FP16, kind="ExternalOutput")
    with tile.TileContext(nc) as tc:
        with ExitStack() as ctx:
            _emit(ctx, tc, nc, x_d, a_d, gb_d, o_d, n_ch)
    nc.compile()
    return nc


def make_core_inputs(inputs, w, gamma, beta, k, n_ch=CPC):
    """Host-side shard prep for core k: padded x slab, banded A matrices, gamma/beta."""
    ch = slice(k * n_ch, (k + 1) * n_ch)
    xk = np.zeros((n_ch, H, N, WP), np.float16)
    xk[:, :, :, 1:1 + W] = np.asarray(inputs[:, ch]).transpose(1, 2, 0, 3)
    wk = np.asarray(w[ch]).astype(np.float32)          # (n_ch, 1, 3, 3)
    ak = np.zeros((n_ch, 3, H, W), np.float32)
    m = np.arange(W)
    for di in range(3):
        # out[m] += w[di, dj] * x[m + di - 1]  ->  A[c, dj, m+di-1, m] = w[c, 0, di, dj]
        sel = (m + di - 1 >= 0) & (m + di - 1 < H)
        ak[:, :, m[sel] + di - 1, m[sel]] = wk[:, 0, di, :][:, :, None]
    ak = np.ascontiguousarray(ak.transpose(2, 0, 1, 3)).astype(np.float16)  # (H, n_ch, 3, W)
    gbk = np.concatenate(
        [np.asarray(gamma[ch]), np.asarray(beta[ch])]
    ).astype(np.float32).reshape(1, 2 * n_ch)
    return {"x": xk, "a": ak, "gb": gbk}


_PROGRAM = None


def kernel(inputs, w, b, gamma, beta):
    global _PROGRAM
    if _PROGRAM is None:
        _PROGRAM = build_program()
    inputs = np.asarray(inputs, np.float32)
    in_maps = [make_core_inputs(inputs, w, gamma, beta, k) for k in range(NCORES)]
    res = bass_utils.run_bass_kernel_spmd(_PROGRAM, in_maps, list(range(NCORES)))
    out = np.empty((N, C, H, W), np.float32)
    for k in range(NCORES):
        # per-core output is (CPC, H, N, W) fp16
        out[:, k * CPC:(k + 1) * CPC] = res.results[k]["o"].transpose(2, 0, 1, 3)
    return out


# revision 8
# speedup vs baseline: 1.3129x; 1.1328x over previous
"""Depthwise 3x3 conv + sync BatchNorm (train mode) + ReLU6 on 8 Trainium2 cores.

Sharding: channels (192) split 24-per-core. Depthwise conv and BN are
per-channel independent, so no cross-core communication is needed.

v4: compound PSUM tiles on top of v3.
  - PSUM pool tiles span 3 banks ([112, 3, 512] fp32); conv matmuls write
    compound [112, 3, 448] outputs (lowering splits into LDWEIGHTS+MATMULxN),
    so LDWEIGHTS drops 24->9 per channel and PSUM drains drop 8->3 per
    channel (each with a free accum_out sum(y), so ACTIVATION_READ_ACCUMULATOR
    drops 8->3 per channel too).
  - sum(y^2): one half on DVE, one half on gpsimd (scalar_tensor_tensor has
    no DVE fast mode - dual tensor reads - so it is split across engines).
  - BN apply + ReLU6: full-channel dual-op tensor_scalar pair on DVE at the
    4x fp16 packed-SBUF rate.
"""

import numpy as np
from contextlib import ExitStack

import concourse.bass as bass
import concourse.mybir as mybir
import concourse.tile as tile
from concourse import bacc, bass_utils

FP32 = mybir.dt.float32
FP16 = mybir.dt.float16
AF = mybir.ActivationFunctionType
ALU = mybir.AluOpType

N, C, H, W = 32, 192, 112, 112
NCORES = 8
CPC = C // NCORES          # 24 channels per core
WP = W + 2                 # zero-padded width only (H handled in banded A)
G = 8                      # image groups (PSUM bank slots) per channel
IPG = N // G               # 4 images per group
NF = IPG * W               # 448 matmul free dim (fp32 PSUM bank limit 512)
NTOT = N * H * W           # BN reduction size per channel
BN_EPS = 1e-5
BANKF = 512                # fp32 elems per PSUM bank per partition
# compound waves: groups per 3-bank PSUM tile
WAVES = ((0, 3), (3, 6), (6, 8))
COMPOUND_MM = False        # compound multi-bank matmul outputs: rejected by
                           # walrus codegen (s3d3_mm_num_elements assert)


def _emit(ctx: ExitStack, tc, nc, x_d, a_d, gb_d, o_d, n_ch):
    a_pool = ctx.enter_context(tc.tile_pool(name="a", bufs=1))
    const_pool = ctx.enter_context(tc.tile_pool(name="const", bufs=1))
    x_pool = ctx.enter_context(tc.tile_pool(name="x", bufs=4))
    y_pool = ctx.enter_context(tc.tile_pool(name="y", bufs=4))
    z_pool = ctx.enter_context(tc.tile_pool(name="z", bufs=2))
    t_pool = ctx.enter_context(tc.tile_pool(name="t", bufs=2))
    q_pool = ctx.enter_context(tc.tile_pool(name="q", bufs=2))
    st_pool = ctx.enter_context(tc.tile_pool(name="st", bufs=3))
    sc_pool = ctx.enter_context(tc.tile_pool(name="sc", bufs=3))
    st = {c: {} for c in range(n_ch)}
    psum_y = ctx.enter_context(tc.tile_pool(name="py", bufs=2, space="PSUM"))
    psum_s = ctx.enter_context(tc.tile_pool(name="ps", bufs=1, space="PSUM"))
    psum_b = ctx.enter_context(tc.tile_pool(name="pb", bufs=1, space="PSUM"))

    a_all = a_pool.tile([H, n_ch, 3, W], FP16)
    nc.sync.dma_start(a_all[:], a_d.ap())
    gb = const_pool.tile([1, 2 * n_ch], FP32)
    nc.sync.dma_start(gb[:], gb_d.ap())
    ones_col = const_pool.tile([H, 1], FP32)   # lhsT for partition collapse
    nc.vector.memset(ones_col[:], 1.0)
    ones_row = const_pool.tile([1, H], FP32)   # lhsT for partition broadcast
    nc.vector.memset(ones_row[:], 1.0)
    eps_t = const_pool.tile([1, 1], FP32)      # BN eps as Sqrt bias operand
    nc.vector.memset(eps_t[:], BN_EPS)

    def emit_conv(c):
        x_t = x_pool.tile([H, N, WP], FP16)
        # alternate input DMAs between the two HWDGE rings (sync / scalar)
        ring = nc.sync if (c % 2 == 0) else nc.scalar
        ring.dma_start(x_t[:], x_d.ap()[c])
        y_sb = y_pool.tile([H, G, NF], FP16)
        # stats: cols 0..2 = per-wave sum(y) (drain accum), 3..4 = sum(y^2)
        stats = st_pool.tile([H, 5], FP32, tag="stats")
        for wv, (g0, g1) in enumerate(WAVES):
            ng = g1 - g0
            pt = psum_y.tile([H, 3, BANKF], FP32, tag="pt")
            for dj in range(3):
                if COMPOUND_MM:
                    nc.tensor.matmul(
                        pt[:, 0:ng, 0:NF],
                        a_all[:, c, dj, :],
                        x_t[:, g0 * IPG:g1 * IPG, dj:dj + W],
                        start=(dj == 0),
                        stop=(dj == 2),
                    )
                else:
                    for j in range(ng):
                        nc.tensor.matmul(
                            pt[:, j, 0:NF],
                            a_all[:, c, dj, :],
                            x_t[:, (g0 + j) * IPG:(g0 + j + 1) * IPG, dj:dj + W],
                            start=(dj == 0),
                            stop=(dj == 2),
                        )
            # compound drain (strided PSUM read) + free per-partition sum(y)
            nc.scalar.activation(
                y_sb[:, g0:g1, :], pt[:, 0:ng, 0:NF], AF.Copy, bias=0.0,
                accum_out=stats[:, wv:wv + 1],
            )
        hg = G // 2
        for h2 in range(2):
            q = q_pool.tile([H, hg * NF], FP16, tag="q")
            yv = y_sb[:, h2 * hg:(h2 + 1) * hg, :].rearrange("p g f -> p (g f)")
            # q = (y*1)*y = y^2 (discarded); accum = sum(y^2).
            # (gpsimd cannot run TensorScalarPtr - engine check fails at codegen)
            nc.vector.scalar_tensor_tensor(
                q[:], yv, 1.0, yv, op0=ALU.mult, op1=ALU.mult,
                accum_out=stats[:, 3 + h2:4 + h2],
            )
        st[c].update(y=y_sb, stats=stats)

    def emit_fin1(c):
        # partition collapse + per-channel scalar chain -> scpair
        stats = st[c]["stats"]
        pst = psum_s.tile([1, 5], FP32, tag="pst")
        nc.tensor.matmul(pst[:], ones_col[:], stats[:])
        sums = sc_pool.tile([1, 2], FP32, tag="sums")
        nc.vector.tensor_reduce(
            sums[:, 0:1], pst[:, 0:3], axis=mybir.AxisListType.X, op=ALU.add
        )
        nc.vector.tensor_reduce(
            sums[:, 1:2], pst[:, 3:5], axis=mybir.AxisListType.X, op=ALU.add
        )
        em = sc_pool.tile([1, 2], FP32, tag="em")   # [mean, E[y^2]]
        nc.vector.tensor_scalar_mul(em[:], sums[:], 1.0 / NTOT)
        m2 = sc_pool.tile([1, 1], FP32, tag="m2")
        nc.vector.tensor_scalar(m2[:], em[:, 0:1], em[:, 0:1], None, op0=ALU.mult)
        varr = sc_pool.tile([1, 1], FP32, tag="varr")
        nc.vector.tensor_scalar(
            varr[:], em[:, 1:2], m2[:], None, op0=ALU.subtract
        )
        std = sc_pool.tile([1, 1], FP32, tag="std")
        nc.scalar.activation(std[:], varr[:], AF.Sqrt, bias=eps_t[:])
        istd = sc_pool.tile([1, 1], FP32, tag="istd")
        nc.vector.reciprocal(istd[:], std[:])
        scpair = sc_pool.tile([1, 2], FP32, tag="scpair")
        nc.vector.tensor_scalar(
            scpair[:, 0:1], istd[:], gb[:, c:c + 1], None, op0=ALU.mult
        )
        msc = sc_pool.tile([1, 1], FP32, tag="msc")
        nc.vector.tensor_scalar(
            msc[:], em[:, 0:1], scpair[:, 0:1], None, op0=ALU.mult
        )
        nc.vector.tensor_scalar(
            scpair[:, 1:2], gb[:, n_ch + c:n_ch + c + 1], msc[:], None,
            op0=ALU.subtract,
        )
        st[c]["scpair"] = scpair

    def emit_fin2(c):
        # broadcast scale'/bias' across partitions (outer product)
        pb = psum_b.tile([H, 2], FP32, tag="pb")
        nc.tensor.matmul(pb[:], ones_row[:], st[c]["scpair"][:])
        bc = sc_pool.tile([H, 2], FP32, tag="bc")
        nc.vector.tensor_copy(bc[:], pb[:])
        st[c]["bc"] = bc

    def emit_out(c):
        y_sb, bc = st[c]["y"], st[c]["bc"]
        z_sb = z_pool.tile([H, G, IPG, W], FP16, tag="z")
        t_sb = t_pool.tile([H, G, IPG, W], FP16, tag="t")
        zf = z_sb.rearrange("p g i w -> p (g i w)")
        tf = t_sb.rearrange("p g i w -> p (g i w)")
        yv = y_sb.rearrange("p g f -> p (g f)")
        # BN apply + ReLU6: full channel, dual-op tensor_scalar at 4x fp16 rate
        nc.vector.tensor_scalar(
            tf, yv, bc[:, 0:1], bc[:, 1:2], op0=ALU.mult, op1=ALU.add
        )
        nc.vector.tensor_scalar(zf, tf, 0.0, 6.0, op0=ALU.max, op1=ALU.min)
        hg = G // 2
        for h2 in range(2):
            # SWDGE ring: keeps the HWDGE rings free for x prefetches
            nc.gpsimd.dma_start(
                o_d.ap()[c].rearrange("h (s n) w -> h s n w", s=2)[:, h2],
                z_sb[:, h2 * hg:(h2 + 1) * hg],
            )

    # software pipeline: PE stream is [fin1(c-1) mm, fin2(c-2) mm, conv(c) mms]
    # so every PE instruction is dep-ready when reached (no in-order stalls)
    for c in range(n_ch):
        if c >= 1:
            emit_fin1(c - 1)
        if c >= 2:
            emit_fin2(c - 2)
        emit_conv(c)
        if c >= 3:
            emit_out(c - 3)
    emit_fin1(n_ch - 1)
    for c in range(max(0, n_ch - 2), n_ch):
        emit_fin2(c)
    for c in range(max(0, n_ch - 3), n_ch):
        emit_out(c)


def build_program(n_ch=CPC, enable_asserts=False):
    nc = bacc.Bacc(
        "TRN2",
        debug=False,
        enable_asserts=enable_asserts,
        target_bir_lowering=False,
        num_devices=NCORES,
    )
    x_d = nc.dram_tensor("x", (n_ch, H, N, WP), FP16, kind="ExternalInput")
    a_d = nc.dram_tensor("a", (H, n_ch, 3, W), FP16, kind="ExternalInput")
    gb_d = nc.dram_tensor("gb", (1, 2 * n_ch), FP32, kind="ExternalInput")
    o_d = nc.dram_tensor("o", (n_ch, H, N, W), FP16, kind="ExternalOutput")
    with tile.TileContext(nc) as tc:
        with ExitStack() as ctx:
            _emit(ctx, tc, nc, x_d, a_d, gb_d, o_d, n_ch)
    nc.compile()
    return nc


def make_core_inputs(inputs, w, gamma, beta, k, n_ch=CPC):
    """Host-side shard prep for core k: padded x slab, banded A matrices, gamma/beta."""
    ch = slice(k * n_ch, (k + 1) * n_ch)
    xk = np.zeros((n_ch, H, N, WP), np.float16)
    xk[:, :, :, 1:1 + W] = np.asarray(inputs[:, ch]).transpose(1, 2, 0, 3)
    wk = np.asarray(w[ch]).astype(np.float32)          # (n_ch, 1, 3, 3)
    ak = np.zeros((n_ch, 3, H, W), np.float32)
    m = np.arange(W)
    for di in range(3):
        # out[m] += w[di, dj] * x[m + di - 1]  ->  A[c, dj, m+di-1, m] = w[c, 0, di, dj]
        sel = (m + di - 1 >= 0) & (m + di - 1 < H)
        ak[:, :, m[sel] + di - 1, m[sel]] = wk[:, 0, di, :][:, :, None]
    ak = np.ascontiguousarray(ak.transpose(2, 0, 1, 3)).astype(np.float16)  # (H, n_ch, 3, W)
    gbk = np.concatenate(
        [np.asarray(gamma[ch]), np.asarray(beta[ch])]
    ).astype(np.float32).reshape(1, 2 * n_ch)
    return {"x": xk, "a": ak, "gb": gbk}


_PROGRAM = None


def kernel(inputs, w, b, gamma, beta):
    global _PROGRAM
    if _PROGRAM is None:
        _PROGRAM = build_program()
    inputs = np.asarray(inputs, np.float32)
    in_maps = [make_core_inputs(inputs, w, gamma, beta, k) for k in range(NCORES)]
    res = bass_utils.run_bass_kernel_spmd(_PROGRAM, in_maps, list(range(NCORES)))
    out = np.empty((N, C, H, W), np.float32)
    for k in range(NCORES):
        # per-core output is (CPC, H, N, W) fp16
        out[:, k * CPC:(k + 1) * CPC] = res.results[k]["o"].transpose(2, 0, 1, 3)
    return out


# revision 12
# speedup vs baseline: 1.3897x; 1.0585x over previous
"""Depthwise 3x3 conv + sync BatchNorm (train mode) + ReLU6 on 8 Trainium2 cores.

Sharding: channels (192) split 24-per-core. Depthwise conv and BN are
per-channel independent, so no cross-core communication is needed.

v5: block-batched BN finalization on top of v4.
  - Channels processed in blocks of 4: one stats tile [112, 4, 5] per block,
    one partition-collapse matmul, one broadcast matmul, and the scalar
    mean/var chain vectorized over the block (tensor_tensor ops on [1,4]),
    cutting the per-channel DVE micro-op tax ~4x.
  - sum(y^2): mostly DVE scalar_tensor_tensor (no fast mode: dual tensor
    reads); on even channels the last quarter runs on ScalarE as
    AF.Square+accum_out to balance engine load.
  - Conv: banded-A matmuls into 3-bank PSUM tiles; compound ScalarE drain
    per wave carries accum_out sum(y). BN apply + ReLU6 = two full-channel
    dual-op tensor_scalar at the 4x fp16 packed-SBUF DVE rate.
"""

import numpy as np
from contextlib import ExitStack

import concourse.bass as bass
import concourse.mybir as mybir
import concourse.tile as tile
from concourse import bacc, bass_utils

FP32 = mybir.dt.float32
FP16 = mybir.dt.float16
AF = mybir.ActivationFunctionType
ALU = mybir.AluOpType

N, C, H, W = 32, 192, 112, 112
NCORES = 8
CPC = C // NCORES          # 24 channels per core
WP = W + 2                 # zero-padded width only (H handled in banded A)
G = 8                      # image groups (PSUM bank slots) per channel
IPG = N // G               # 4 images per group
NF = IPG * W               # 448 matmul free dim (fp32 PSUM bank limit 512)
NTOT = N * H * W           # BN reduction size per channel
BN_EPS = 1e-5
BANKF = 512                # fp32 elems per PSUM bank per partition
WAVES = ((0, 3), (3, 6), (6, 8))   # groups per 3-bank PSUM tile
CB = 4                     # channels per finalization block
NB = CPC // CB


def _emit(ctx: ExitStack, tc, nc, x_d, a_d, gb_d, o_d, n_ch):
    a_pool = ctx.enter_context(tc.tile_pool(name="a", bufs=1))
    const_pool = ctx.enter_context(tc.tile_pool(name="const", bufs=1))
    x_pool = ctx.enter_context(tc.tile_pool(name="x", bufs=4))
    y_pool = ctx.enter_context(tc.tile_pool(name="y", bufs=10))
    z_pool = ctx.enter_context(tc.tile_pool(name="z", bufs=3))
    t_pool = ctx.enter_context(tc.tile_pool(name="t", bufs=2))
    q_pool = ctx.enter_context(tc.tile_pool(name="q", bufs=2))
    st_pool = ctx.enter_context(tc.tile_pool(name="st", bufs=2))
    sc_pool = ctx.enter_context(tc.tile_pool(name="sc", bufs=2))
    bl = {b: {} for b in range(NB)}
    psum_y = ctx.enter_context(tc.tile_pool(name="py", bufs=2, space="PSUM"))
    psum_s = ctx.enter_context(tc.tile_pool(name="ps", bufs=1, space="PSUM"))
    psum_b = ctx.enter_context(tc.tile_pool(name="pb", bufs=1, space="PSUM"))

    a_all = a_pool.tile([H, n_ch, 3, W], FP16)
    nc.sync.dma_start(a_all[:], a_d.ap())
    gb = const_pool.tile([1, 2 * n_ch], FP32)
    nc.sync.dma_start(gb[:], gb_d.ap())
    ones_col = const_pool.tile([H, 1], FP32)   # lhsT for partition collapse
    nc.vector.memset(ones_col[:], 1.0)
    ones_row = const_pool.tile([1, H], FP32)   # lhsT for partition broadcast
    nc.vector.memset(ones_row[:], 1.0)
    eps_t = const_pool.tile([1, 1], FP32)      # BN eps as Sqrt bias operand
    nc.vector.memset(eps_t[:], BN_EPS)

    def emit_conv(c):
        b, cb = divmod(c, CB)
        x_t = x_pool.tile([H, N, WP], FP16)
        # alternate input DMAs between the two HWDGE rings (sync / scalar)
        ring = nc.sync if (c % 2 == 0) else nc.scalar
        ring.dma_start(x_t[:], x_d.ap()[c])
        y_sb = y_pool.tile([H, G, NF], FP16)
        if cb == 0:
            # block stats: [ch-in-block, 3 wave sums(y) | 2 sums(y^2)]
            stats_blk = st_pool.tile([H, CB, 5], FP32, tag="stats")
            bl[b]["stats"] = stats_blk
            bl[b]["y"] = {}
        stats = bl[b]["stats"]
        for wv, (g0, g1) in enumerate(WAVES):
            ng = g1 - g0
            pt = psum_y.tile([H, 3, BANKF], FP32, tag="pt")
            for dj in range(3):
                for j in range(ng):
                    nc.tensor.matmul(
                        pt[:, j, 0:NF],
                        a_all[:, c, dj, :],
                        x_t[:, (g0 + j) * IPG:(g0 + j + 1) * IPG, dj:dj + W],
                        start=(dj == 0),
                        stop=(dj == 2),
                    )
            # compound drain (strided PSUM read) + free per-partition sum(y)
            nc.scalar.activation(
                y_sb[:, g0:g1, :], pt[:, 0:ng, 0:NF], AF.Copy, bias=0.0,
                accum_out=stats[:, cb, wv:wv + 1],
            )
        # sum(y^2): q = (y*1)*y discarded, accum_out = sum.  On even channels
        # the last quarter runs on ScalarE (AF.Square+accum) for balance.
        yf = y_sb.rearrange("p g f -> p (g f)")
        FT = G * NF
        if c % 2 == 0:
            q = q_pool.tile([H, FT - FT // 4], FP16, tag="qa")
            nc.vector.scalar_tensor_tensor(
                q[:], yf[:, 0:FT - FT // 4], 1.0, yf[:, 0:FT - FT // 4],
                op0=ALU.mult, op1=ALU.mult, accum_out=stats[:, cb, 3:4],
            )
            q2 = q_pool.tile([H, FT // 4], FP16, tag="q2")
            nc.scalar.activation(
                q2[:], yf[:, FT - FT // 4:FT], AF.Square,
                accum_out=stats[:, cb, 4:5],
            )
        else:
            for h2 in range(2):
                q = q_pool.tile([H, FT // 2], FP16, tag="qh")
                nc.vector.scalar_tensor_tensor(
                    q[:], yf[:, h2 * FT // 2:(h2 + 1) * FT // 2], 1.0,
                    yf[:, h2 * FT // 2:(h2 + 1) * FT // 2],
                    op0=ALU.mult, op1=ALU.mult,
                    accum_out=stats[:, cb, 3 + h2:4 + h2],
                )
        bl[b]["y"][cb] = y_sb

    def emit_fin(b):
        # block partition collapse + vectorized mean/var chain -> bc block
        stats = bl[b]["stats"]
        pst = psum_s.tile([1, CB, 5], FP32, tag="pst")
        nc.tensor.matmul(pst[:], ones_col[:], stats[:])
        sums = sc_pool.tile([1, 2, CB], FP32, tag="sums")
        nc.vector.tensor_reduce(
            sums[:, 0, :], pst[:, :, 0:3].rearrange("p c w -> p c w"),
            axis=mybir.AxisListType.X, op=ALU.add,
        )
        nc.vector.tensor_reduce(
            sums[:, 1, :], pst[:, :, 3:5].rearrange("p c w -> p c w"),
            axis=mybir.AxisListType.X, op=ALU.add,
        )
        em = sc_pool.tile([1, 2, CB], FP32, tag="em")   # [mean | E[y^2]]
        nc.vector.tensor_scalar_mul(
            em.rearrange("p a c -> p (a c)"), sums.rearrange("p a c -> p (a c)"),
            1.0 / NTOT,
        )
        varr = sc_pool.tile([1, CB], FP32, tag="varr")
        nc.vector.tensor_tensor(varr[:], em[:, 0, :], em[:, 0, :], op=ALU.mult)
        nc.vector.tensor_tensor(varr[:], em[:, 1, :], varr[:], op=ALU.subtract)
        std = sc_pool.tile([1, CB], FP32, tag="std")
        nc.scalar.activation(std[:], varr[:], AF.Sqrt, bias=eps_t[:])
        # scb: row 0 = scale' = gamma*istd, row 1 = bias' = beta - mean*scale'
        scb = sc_pool.tile([1, 2, CB], FP32, tag="scb")
        nc.vector.reciprocal(scb[:, 0, :], std[:])
        nc.vector.tensor_tensor(
            scb[:, 0, :], scb[:, 0, :], gb[:, b * CB:(b + 1) * CB], op=ALU.mult
        )
        msc = sc_pool.tile([1, CB], FP32, tag="msc")
        nc.vector.tensor_tensor(msc[:], em[:, 0, :], scb[:, 0, :], op=ALU.mult)
        nc.vector.tensor_tensor(
            scb[:, 1, :], gb[:, n_ch + b * CB:n_ch + (b + 1) * CB], msc[:],
            op=ALU.subtract,
        )
        # broadcast scale'/bias' across partitions (outer product)
        pb = psum_b.tile([H, 2 * CB], FP32, tag="pb")
        nc.tensor.matmul(pb[:], ones_row[:], scb.rearrange("p a c -> p (a c)"))
        bc = sc_pool.tile([H, 2, CB], FP32, tag="bc")
        nc.vector.tensor_copy(bc.rearrange("p a c -> p (a c)"), pb[:])
        bl[b]["bc"] = bc

    def emit_out(b, cb):
        y_sb, bc = bl[b]["y"][cb], bl[b]["bc"]
        c = b * CB + cb
        z_sb = z_pool.tile([H, G, IPG, W], FP16, tag="z")
        t_sb = t_pool.tile([H, G, IPG, W], FP16, tag="t")
        zf = z_sb.rearrange("p g i w -> p (g i w)")
        tf = t_sb.rearrange("p g i w -> p (g i w)")
        yv = y_sb.rearrange("p g f -> p (g f)")
        # BN apply + ReLU6: full channel, dual-op tensor_scalar at 4x fp16 rate
        nc.vector.tensor_scalar(
            tf, yv, bc[:, 0, cb:cb + 1], bc[:, 1, cb:cb + 1],
            op0=ALU.mult, op1=ALU.add,
        )
        nc.vector.tensor_scalar(zf, tf, 0.0, 6.0, op0=ALU.max, op1=ALU.min)
        hg = G // 2
        for h2 in range(2):
            # SWDGE ring: keeps the HWDGE rings free for x prefetches
            nc.gpsimd.dma_start(
                o_d.ap()[c].rearrange("h (s n) w -> h s n w", s=2)[:, h2],
                z_sb[:, h2 * hg:(h2 + 1) * hg],
            )

    # software pipeline over blocks of CB channels:
    #   conv(block b) interleaved with fin(b-1) and out(b-1 channels)
    for b in range(NB):
        for cb in range(CB):
            emit_conv(b * CB + cb)
            if b >= 1:
                if cb == 0:
                    emit_fin(b - 1)
                else:
                    emit_out(b - 1, cb - 1)
        if b >= 1:
            emit_out(b - 1, CB - 1)
    emit_fin(NB - 1)
    for cb in range(CB):
        emit_out(NB - 1, cb)


def build_program(n_ch=CPC, enable_asserts=False):
    nc = bacc.Bacc(
        "TRN2",
        debug=False,
        enable_asserts=enable_asserts,
        target_bir_lowering=False,
        num_devices=NCORES,
    )
    x_d = nc.dram_tensor("x", (n_ch, H, N, WP), FP16, kind="ExternalInput")
    a_d = nc.dram_tensor("a", (H, n_ch, 3, W), FP16, kind="ExternalInput")
    gb_d = nc.dram_tensor("gb", (1, 2 * n_ch), FP32, kind="ExternalInput")
    o_d = nc.dram_tensor("o", (n_ch, H, N, W), FP16, kind="ExternalOutput")
    with tile.TileContext(nc) as tc:
        with ExitStack() as ctx:
            _emit(ctx, tc, nc, x_d, a_d, gb_d, o_d, n_ch)
    nc.compile()
    return nc


def make_core_inputs(inputs, w, gamma, beta, k, n_ch=CPC):
    """Host-side shard prep for core k: padded x slab, banded A matrices, gamma/beta."""
    ch = slice(k * n_ch, (k + 1) * n_ch)
    xk = np.zeros((n_ch, H, N, WP), np.float16)
    xk[:, :, :, 1:1 + W] = np.asarray(inputs[:, ch]).transpose(1, 2, 0, 3)
    wk = np.asarray(w[ch]).astype(np.float32)          # (n_ch, 1, 3, 3)
    ak = np.zeros((n_ch, 3, H, W), np.float32)
    m = np.arange(W)
    for di in range(3):
        # out[m] += w[di, dj] * x[m + di - 1]  ->  A[c, dj, m+di-1, m] = w[c, 0, di, dj]
        sel = (m + di - 1 >= 0) & (m + di - 1 < H)
        ak[:, :, m[sel] + di - 1, m[sel]] = wk[:, 0, di, :][:, :, None]
    ak = np.ascontiguousarray(ak.transpose(2, 0, 1, 3)).astype(np.float16)  # (H, n_ch, 3, W)
    gbk = np.concatenate(
        [np.asarray(gamma[ch]), np.asarray(beta[ch])]
    ).astype(np.float32).reshape(1, 2 * n_ch)
    return {"x": xk, "a": ak, "gb": gbk}


_PROGRAM = None


def kernel(inputs, w, b, gamma, beta):
    global _PROGRAM
    if _PROGRAM is None:
        _PROGRAM = build_program()
    inputs = np.asarray(inputs, np.float32)
    in_maps = [make_core_inputs(inputs, w, gamma, beta, k) for k in range(NCORES)]
    res = bass_utils.run_bass_kernel_spmd(_PROGRAM, in_maps, list(range(NCORES)))
    out = np.empty((N, C, H, W), np.float32)
    for k in range(NCORES):
        # per-core output is (CPC, H, N, W) fp16
        out[:, k * CPC:(k + 1) * CPC] = res.results[k]["o"].transpose(2, 0, 1, 3)
    return out


# revision 21
# speedup vs baseline: 1.4038x; 1.0102x over previous
"""Depthwise 3x3 conv + sync BatchNorm (train mode) + ReLU6 on 8 Trainium2 cores.

Sharding: channels (192) split 24-per-core. Depthwise conv and BN are
per-channel independent, so no cross-core communication is needed.

v5: block-batched BN finalization on top of v4.
  - Channels processed in blocks of 4: one stats tile [112, 4, 5] per block,
    one partition-collapse matmul, one broadcast matmul, and the scalar
    mean/var chain vectorized over the block (tensor_tensor ops on [1,4]),
    cutting the per-channel DVE micro-op tax ~4x.
  - sum(y^2): mostly DVE scalar_tensor_tensor (no fast mode: dual tensor
    reads); on even channels the last quarter runs on ScalarE as
    AF.Square+accum_out to balance engine load.
  - Conv: banded-A matmuls into 3-bank PSUM tiles; compound ScalarE drain
    per wave carries accum_out sum(y). BN apply + ReLU6 = two full-channel
    dual-op tensor_scalar at the 4x fp16 packed-SBUF DVE rate.
"""

import numpy as np
from contextlib import ExitStack

import concourse.bass as bass
import concourse.mybir as mybir
import concourse.tile as tile
from concourse import bacc, bass_utils

FP32 = mybir.dt.float32
FP16 = mybir.dt.float16
AF = mybir.ActivationFunctionType
ALU = mybir.AluOpType

N, C, H, W = 32, 192, 112, 112
NCORES = 8
CPC = C // NCORES          # 24 channels per core
WP = W + 2                 # zero-padded width only (H handled in banded A)
G = 8                      # image groups (PSUM bank slots) per channel
IPG = N // G               # 4 images per group
NF = IPG * W               # 448 matmul free dim (fp32 PSUM bank limit 512)
NTOT = N * H * W           # BN reduction size per channel
BN_EPS = 1e-5
BANKF = 512                # fp32 elems per PSUM bank per partition
WAVES = ((0, 3), (3, 6), (6, 8))   # groups per 3-bank PSUM tile
CB = 4                     # channels per finalization block
NB = CPC // CB


def _emit(ctx: ExitStack, tc, nc, x_d, a_d, gb_d, o_d, n_ch):
    a_pool = ctx.enter_context(tc.tile_pool(name="a", bufs=1))
    const_pool = ctx.enter_context(tc.tile_pool(name="const", bufs=1))
    x_pool = ctx.enter_context(tc.tile_pool(name="x", bufs=5))
    y_pool = ctx.enter_context(tc.tile_pool(name="y", bufs=9))
    z_pool = ctx.enter_context(tc.tile_pool(name="z", bufs=2))
    t_pool = ctx.enter_context(tc.tile_pool(name="t", bufs=2))
    q_pool = ctx.enter_context(tc.tile_pool(name="q", bufs=2))
    st_pool = ctx.enter_context(tc.tile_pool(name="st", bufs=2))
    sc_pool = ctx.enter_context(tc.tile_pool(name="sc", bufs=2))
    bl = {b: {} for b in range(NB)}
    psum_y = ctx.enter_context(tc.tile_pool(name="py", bufs=2, space="PSUM"))
    psum_s = ctx.enter_context(tc.tile_pool(name="ps", bufs=1, space="PSUM"))
    psum_b = ctx.enter_context(tc.tile_pool(name="pb", bufs=1, space="PSUM"))

    # A matrices uploaded in per-block chunks so the first conv does not wait
    # on the whole 1.7MB tensor; alternate rings for overlap.
    a_blk = []
    for b in range(NB):
        ab = a_pool.tile([H, CB, 3, W], FP16, tag=f"ab{b}")
        ring = nc.sync if (b % 2 == 0) else nc.scalar
        ring.dma_start(ab[:], a_d.ap()[:, b * CB:(b + 1) * CB])
        a_blk.append(ab)
    gb = const_pool.tile([1, 2 * n_ch], FP32)
    nc.sync.dma_start(gb[:], gb_d.ap())
    ones_col = const_pool.tile([H, 1], FP32)   # lhsT for partition collapse
    nc.vector.memset(ones_col[:], 1.0)
    ones_row = const_pool.tile([1, H], FP32)   # lhsT for partition broadcast
    nc.vector.memset(ones_row[:], 1.0)
    eps_t = const_pool.tile([1, 1], FP32)      # BN eps as Sqrt bias operand
    nc.vector.memset(eps_t[:], BN_EPS)

    def emit_conv(c):
        b, cb = divmod(c, CB)
        x_t = x_pool.tile([H, N, WP], FP16)
        # alternate input DMAs between the two HWDGE rings (sync / scalar)
        ring = nc.sync if (c % 2 == 0) else nc.scalar
        ring.dma_start(x_t[:], x_d.ap()[c])
        y_sb = y_pool.tile([H, G, NF], FP16)
        if cb == 0:
            # block stats: [ch-in-block, 3 wave sums(y) | 2 sums(y^2)]
            stats_blk = st_pool.tile([H, CB, 5], FP32, tag="stats")
            bl[b]["stats"] = stats_blk
            bl[b]["y"] = {}
        stats = bl[b]["stats"]
        for wv, (g0, g1) in enumerate(WAVES):
            ng = g1 - g0
            pt = psum_y.tile([H, 3, BANKF], FP32, tag="pt")
            for dj in range(3):
                for j in range(ng):
                    nc.tensor.matmul(
                        pt[:, j, 0:NF],
                        a_blk[b][:, cb, dj, :],
                        x_t[:, (g0 + j) * IPG:(g0 + j + 1) * IPG, dj:dj + W],
                        start=(dj == 0),
                        stop=(dj == 2),
                    )
            # compound drain (strided PSUM read) + free per-partition sum(y)
            nc.scalar.activation(
                y_sb[:, g0:g1, :], pt[:, 0:ng, 0:NF], AF.Copy, bias=0.0,
                accum_out=stats[:, cb, wv:wv + 1],
            )
        # sum(y^2): q = (y*1)*y discarded, accum_out = sum.  On even channels
        # the last quarter runs on ScalarE (AF.Square+accum) for balance.
        yf = y_sb.rearrange("p g f -> p (g f)")
        FT = G * NF
        if c % 2 == 0:
            q = q_pool.tile([H, FT - FT // 4], FP16, tag="qa")
            nc.vector.scalar_tensor_tensor(
                q[:], yf[:, 0:FT - FT // 4], 1.0, yf[:, 0:FT - FT // 4],
                op0=ALU.mult, op1=ALU.mult, accum_out=stats[:, cb, 3:4],
            )
            q2 = q_pool.tile([H, FT // 4], FP16, tag="q2")
            nc.scalar.activation(
                q2[:], yf[:, FT - FT // 4:FT], AF.Square,
                accum_out=stats[:, cb, 4:5],
            )
        else:
            for h2 in range(2):
                q = q_pool.tile([H, FT // 2], FP16, tag="qh")
                nc.vector.scalar_tensor_tensor(
                    q[:], yf[:, h2 * FT // 2:(h2 + 1) * FT // 2], 1.0,
                    yf[:, h2 * FT // 2:(h2 + 1) * FT // 2],
                    op0=ALU.mult, op1=ALU.mult,
                    accum_out=stats[:, cb, 3 + h2:4 + h2],
                )
        bl[b]["y"][cb] = y_sb

    def emit_fin_pst(b):
        # block partition collapse (PE) -> pst
        stats = bl[b]["stats"]
        pst = psum_s.tile([1, CB, 5], FP32, tag="pst")
        nc.tensor.matmul(pst[:], ones_col[:], stats[:])
        bl[b]["pst"] = pst

    def emit_fin_chain(b):
        # vectorized mean/var chain + broadcast -> bc block
        pst = bl[b]["pst"]
        sums = sc_pool.tile([1, 2, CB], FP32, tag="sums")
        nc.vector.tensor_reduce(
            sums[:, 0, :], pst[:, :, 0:3].rearrange("p c w -> p c w"),
            axis=mybir.AxisListType.X, op=ALU.add,
        )
        nc.vector.tensor_reduce(
            sums[:, 1, :], pst[:, :, 3:5].rearrange("p c w -> p c w"),
            axis=mybir.AxisListType.X, op=ALU.add,
        )
        em = sc_pool.tile([1, 2, CB], FP32, tag="em")   # [mean | E[y^2]]
        nc.vector.tensor_scalar_mul(
            em.rearrange("p a c -> p (a c)"), sums.rearrange("p a c -> p (a c)"),
            1.0 / NTOT,
        )
        varr = sc_pool.tile([1, CB], FP32, tag="varr")
        nc.vector.tensor_tensor(varr[:], em[:, 0, :], em[:, 0, :], op=ALU.mult)
        nc.vector.tensor_tensor(varr[:], em[:, 1, :], varr[:], op=ALU.subtract)
        std = sc_pool.tile([1, CB], FP32, tag="std")
        nc.scalar.activation(std[:], varr[:], AF.Sqrt, bias=eps_t[:])
        # scb: row 0 = scale' = gamma*istd, row 1 = bias' = beta - mean*scale'
        scb = sc_pool.tile([1, 2, CB], FP32, tag="scb")
        nc.vector.reciprocal(scb[:, 0, :], std[:])
        nc.vector.tensor_tensor(
            scb[:, 0, :], scb[:, 0, :], gb[:, b * CB:(b + 1) * CB], op=ALU.mult
        )
        msc = sc_pool.tile([1, CB], FP32, tag="msc")
        nc.vector.tensor_tensor(msc[:], em[:, 0, :], scb[:, 0, :], op=ALU.mult)
        nc.vector.tensor_tensor(
            scb[:, 1, :], gb[:, n_ch + b * CB:n_ch + (b + 1) * CB], msc[:],
            op=ALU.subtract,
        )
        # broadcast scale'/bias' across partitions (outer product)
        pb = psum_b.tile([H, 2 * CB], FP32, tag="pb")
        nc.tensor.matmul(pb[:], ones_row[:], scb.rearrange("p a c -> p (a c)"))
        bc = sc_pool.tile([H, 2, CB], FP32, tag="bc")
        nc.vector.tensor_copy(bc.rearrange("p a c -> p (a c)"), pb[:])
        bl[b]["bc"] = bc

    def emit_out(b, cb):
        y_sb, bc = bl[b]["y"][cb], bl[b]["bc"]
        c = b * CB + cb
        z_sb = z_pool.tile([H, G, IPG, W], FP16, tag="z")
        t_sb = t_pool.tile([H, G, IPG, W], FP16, tag="t")
        hg = G // 2
        # BN apply + ReLU6: dual-op tensor_scalar pair at 4x fp16 rate.
        # Full-channel normally; per-half for the last channel so the final
        # output DMA starts earlier (shorter kernel tail).
        nhalf = 2 if c == n_ch - 1 else 1
        for p in range(nhalf):
            sl = slice(p * G // nhalf, (p + 1) * G // nhalf)
            zf = z_sb[:, sl].rearrange("p g i w -> p (g i w)")
            tf = t_sb[:, sl].rearrange("p g i w -> p (g i w)")
            yv = y_sb[:, sl, :].rearrange("p g f -> p (g f)")
            nc.vector.tensor_scalar(
                tf, yv, bc[:, 0, cb:cb + 1], bc[:, 1, cb:cb + 1],
                op0=ALU.mult, op1=ALU.add,
            )
            nc.vector.tensor_scalar(zf, tf, 0.0, 6.0, op0=ALU.max, op1=ALU.min)
            for h2 in range(p * (2 // nhalf), (p + 1) * (2 // nhalf)):
                # SWDGE ring: keeps the HWDGE rings free for x prefetches
                nc.gpsimd.dma_start(
                    o_d.ap()[c].rearrange("h (s n) w -> h s n w", s=2)[:, h2],
                    z_sb[:, h2 * hg:(h2 + 1) * hg],
                )

    # software pipeline over blocks of CB channels; fin stages staggered one
    # conv-slot apart so stats-tail and chain latency hide behind convs
    for b in range(NB):
        for cb in range(CB):
            emit_conv(b * CB + cb)
            if b >= 1:
                if cb == 0:
                    emit_fin_pst(b - 1)
                elif cb == 1:
                    emit_fin_chain(b - 1)
                else:
                    emit_out(b - 1, cb - 2)
        if b >= 1:
            emit_out(b - 1, 2)
            emit_out(b - 1, 3)
    emit_fin_pst(NB - 1)
    emit_fin_chain(NB - 1)
    for cb in range(CB):
        emit_out(NB - 1, cb)


def build_program(n_ch=CPC, enable_asserts=False):
    nc = bacc.Bacc(
        "TRN2",
        debug=False,
        enable_asserts=enable_asserts,
        target_bir_lowering=False,
        num_devices=NCORES,
    )
    x_d = nc.dram_tensor("x", (n_ch, H, N, WP), FP16, kind="ExternalInput")
    a_d = nc.dram_tensor("a", (H, n_ch, 3, W), FP16, kind="ExternalInput")
    gb_d = nc.dram_tensor("gb", (1, 2 * n_ch), FP32, kind="ExternalInput")
    o_d = nc.dram_tensor("o", (n_ch, H, N, W), FP16, kind="ExternalOutput")
    with tile.TileContext(nc) as tc:
        with ExitStack() as ctx:
            _emit(ctx, tc, nc, x_d, a_d, gb_d, o_d, n_ch)
    nc.compile()
    return nc


def make_core_inputs(inputs, w, gamma, beta, k, n_ch=CPC):
    """Host-side shard prep for core k: padded x slab, banded A matrices, gamma/beta."""
    ch = slice(k * n_ch, (k + 1) * n_ch)
    xk = np.zeros((n_ch, H, N, WP), np.float16)
    xk[:, :, :, 1:1 + W] = np.asarray(inputs[:, ch]).transpose(1, 2, 0, 3)
    wk = np.asarray(w[ch]).astype(np.float32)          # (n_ch, 1, 3, 3)
    ak = np.zeros((n_ch, 3, H, W), np.float32)
    m = np.arange(W)
    for di in range(3):
        # out[m] += w[di, dj] * x[m + di - 1]  ->  A[c, dj, m+di-1, m] = w[c, 0, di, dj]
        sel = (m + di - 1 >= 0) & (m + di - 1 < H)
        ak[:, :, m[sel] + di - 1, m[sel]] = wk[:, 0, di, :][:, :, None]
    ak = np.ascontiguousarray(ak.transpose(2, 0, 1, 3)).astype(np.float16)  # (H, n_ch, 3, W)
    gbk = np.concatenate(
        [np.asarray(gamma[ch]), np.asarray(beta[ch])]
    ).astype(np.float32).reshape(1, 2 * n_ch)
    return {"x": xk, "a": ak, "gb": gbk}


_PROGRAM = None


def kernel(inputs, w, b, gamma, beta):
    global _PROGRAM
    if _PROGRAM is None:
        _PROGRAM = build_program()
    inputs = np.asarray(inputs, np.float32)
    in_maps = [make_core_inputs(inputs, w, gamma, beta, k) for k in range(NCORES)]
    res = bass_utils.run_bass_kernel_spmd(_PROGRAM, in_maps, list(range(NCORES)))
    out = np.empty((N, C, H, W), np.float32)
    for k in range(NCORES):
        # per-core output is (CPC, H, N, W) fp16
        out[:, k * CPC:(k + 1) * CPC] = res.results[k]["o"].transpose(2, 0, 1, 3)
    return out
